# revision 5
# baseline (speedup 1.0000x reference)
"""AdaptiveGridKANLayer on 8 TRN2 NeuronCores.

out[b,o] = sum_i sum_g exp(-((x[b,i]-c_g)/w)^2) * coeffs[o,i,g]
         + sum_i silu(x[b,i]) * base_w[o,i]

B=65536, in=out=128, G=8, centers = linspace(-1,1,8), w = 2/7.

Strategy (data-parallel over batch, weights replicated):
- Host: transpose x to feature-major [128, B], shard columns 8 ways; fold the
  Gaussian factorization constants e^(7g-g^2) into the coeffs.
- Device, per core (u = (x+1)/w): basis_g = e^(-(u-g)^2) = p * s^g * const
  with p = exp(-u^2) (ScalarE Square+Exp), s = exp(7x) (ScalarE Exp).
  VectorE builds the power chain t_g = t_{g-1} * s (bf16 2x-mode);
  TensorE contracts 8 Gaussian K-tiles g-major per chunk, plus one silu
  K-tile per 512-col psum tile.
- PSUM plan (8 banks, 16 x 512-col f32 tiles):
  tiles 0-7  close at g=7 (chain-paced, early), drain via ScalarE copies
             that ride in the exp-phase pacing slack -> "out";
  tiles 8-15 stay open and are closed by their silu matmul (which can only
             exist after the one exp->silu table switch), drained late;
  tiles 0-7's silu contribution runs afterwards as single-MM groups in the
  banks freed by tiles 8-15, drained to a separate partial "outs" that the
  host adds during the unshard.
- All scalar exp-table work (s, q, t0) runs before the single table switch;
  silu acts after.  Late psum drains are split between ScalarE and VectorE
  tails.  The PE clock gate is warmed by memset-fed matmuls.
"""

import numpy as np

BATCH = 65536
GRID = 8
NCORES = 8
BLOC = BATCH // NCORES  # 8192 batch columns per core
FDP = 512  # psum tile (one bank)
NSUB = BLOC // FDP  # 16 psum tiles
G1 = 8  # tiles 0..G1-1 close early at g=7; the rest close via silu MM
W = 2.0 / (GRID - 1)

# elementwise chunks (fixed 2048-wide tiles) split into pieces; narrow first
# pieces for early start, narrow tail pieces for a short drain.
FDE = 2048
CHUNKS = [[512, 512, 1024], [2048], [2048], [1024, 1024]]

_NC = None


def _build():
    import concourse.mybir as mybir
    from concourse import bacc
    from concourse.tile import TileContext, add_dep_helper

    AF = mybir.ActivationFunctionType
    bf16 = mybir.dt.bfloat16
    f32 = mybir.dt.float32

    nc = bacc.Bacc("TRN2", num_devices=NCORES)
    cst = nc.alloc_sbuf_tensor("const-float32-bias-c", [128, 1], f32)
    nc.gpsimd.memset(cst.ap(), 1.0 / W)
    nc.const_aps.aps[(f32, 1.0 / W)] = cst.ap()
    nc.all_engine_barrier()
    xt = nc.dram_tensor("xt", [128, BLOC], f32, kind="ExternalInput").ap()
    wt = nc.dram_tensor("wt", [128, 9 * 128], bf16, kind="ExternalInput").ap()
    out = nc.dram_tensor("out", [128, BLOC], bf16, kind="ExternalOutput").ap()
    outs = nc.dram_tensor(
        "outs", [128, G1 * FDP], bf16, kind="ExternalOutput"
    ).ap()

    with TileContext(nc) as tc:
        with (
            tc.tile_pool(name="const", bufs=1) as cpool,
            tc.tile_pool(name="work", bufs=2) as wpool,
            tc.tile_pool(name="obuf", bufs=8) as opool,
            tc.tile_pool(name="psum", bufs=8, space="PSUM") as ppool,
        ):
            # Dummy activation with no DMA deps: forces the exp_and_others
            # ACT table load into the preamble.
            warm_act = cpool.tile([128, 1], f32, name="warm_act")
            nc.vector.memset(warm_act[:], 0.0)
            nc.scalar.activation(warm_act[:], warm_act[:], AF.Exp, scale=1.0)

            # PE HAM clock warm: memset-fed matmuls (no DMA deps).
            wm_s = cpool.tile([128, 128], bf16, name="wm_s")
            wm_m = cpool.tile([128, FDP], bf16, name="wm_m")
            nc.vector.memset(wm_s[:], 0.25)
            nc.vector.memset(wm_m[:], 0.25)
            warm_ps = ppool.tile([128, FDP], f32, name="warm_ps", tag="psum")
            for _ in range(12):
                nc.tensor.matmul(
                    warm_ps[:], wm_s[:], wm_m[:], start=True, stop=True
                )

            # x stream (and weights after the second piece) on the sync queue.
            w_sb = cpool.tile([128, 9, 128], bf16, name="w_sb")
            x_all = cpool.tile([128, BLOC], f32, name="x_all")
            lo = 0
            for i, wd in enumerate([p for ch in CHUNKS for p in ch]):
                nc.sync.dma_start(x_all[:, lo : lo + wd], xt[:, lo : lo + wd])
                lo += wd
                if i == 1:
                    nc.sync.dma_start(
                        w_sb[:], wt.rearrange("p (g o) -> p g o", g=9)
                    )

            # ---- exp phase (scalar) + chain (vector) + gauss MMs (tensor),
            # piece by piece; gen-1 tiles close at g=7 and their ScalarE
            # drain copies are interleaved into the scalar stream.
            psums = [None] * NSUB
            obufs = [None] * NSUB
            last_exp_op = None
            first_silu_op = None

            def emit_copy(k, engine):
                ob = opool.tile([128, FDP], bf16, tag="ob", name=f"ob_{k}")
                obufs[k] = ob
                if engine == "s":
                    nc.scalar.copy(ob[:], psums[k][:])
                else:
                    nc.vector.tensor_copy(ob[:], psums[k][:])
                nc.scalar.dma_start(out[:, k * FDP : (k + 1) * FDP], ob[:])

            lo = 0
            for c, pieces in enumerate(CHUNKS):
                s = wpool.tile([128, FDE], bf16, tag="s", name=f"s_{c}")
                q = wpool.tile([128, FDE], f32, tag="q", name=f"q_{c}")
                tg = [
                    wpool.tile([128, FDE], bf16, tag=f"t{g}", name=f"t{g}_{c}")
                    for g in range(GRID)
                ]
                off = 0
                for wd in pieces:
                    hs = slice(off, off + wd)
                    xc = x_all[:, lo + off : lo + off + wd]
                    nc.scalar.activation(s[:, hs], xc, AF.Exp, scale=2.0 / W)
                    nc.scalar.activation(
                        q[:, hs], xc, AF.Square, bias=1.0 / W, scale=1.0 / W
                    )
                    last_exp_op = nc.scalar.activation(
                        tg[0][:, hs], q[:, hs], AF.Exp, scale=-1.0
                    )
                    for g in range(1, GRID):
                        nc.vector.tensor_mul(
                            tg[g][:, hs], tg[g - 1][:, hs], s[:, hs]
                        )
                    # tensor: g-major over this piece's psum tiles
                    ntile = wd // FDP
                    k0 = (lo + off) // FDP
                    for k in range(k0, k0 + ntile):
                        psums[k] = ppool.tile(
                            [128, FDP], f32, tag="psum", name=f"psum_{k}"
                        )
                    for g in range(GRID):
                        for k in range(k0, k0 + ntile):
                            mlo = off + (k - k0) * FDP
                            nc.tensor.matmul(
                                psums[k][:],
                                w_sb[:, g, :],
                                tg[g][:, mlo : mlo + FDP],
                                start=(g == 0),
                                stop=(g == GRID - 1 and k < G1),
                            )
                    # early drains for gen-1 tiles (ride in scalar slack)
                    for k in range(k0, min(k0 + ntile, G1)):
                        emit_copy(k, "s")
                    off += wd
                lo += FDE

            # ---- silu phase: one table switch, acts for gen-2 cols first.
            silu_sb = cpool.tile([128, BLOC], bf16, name="silu_sb")
            silu_ops = [None] * NSUB
            order = list(range(G1, NSUB)) + list(range(G1))
            for j, k in enumerate(order):
                ks = slice(k * FDP, (k + 1) * FDP)
                op = nc.scalar.activation(silu_sb[:, ks], x_all[:, ks], AF.Silu)
                silu_ops[k] = op
                if j == 0:
                    first_silu_op = op
                    add_dep_helper(
                        op.ins, last_exp_op.ins, True, "table phase order"
                    )

            # gen-2 silu MMs close their groups; drains split scalar/vector.
            for k in range(G1, NSUB):
                ks = slice(k * FDP, (k + 1) * FDP)
                nc.tensor.matmul(
                    psums[k][:],
                    w_sb[:, 8, :],
                    silu_sb[:, ks],
                    start=False,
                    stop=True,
                )
            for j, k in enumerate(range(G1, NSUB)):
                emit_copy(k, "v" if j % 2 == 0 else "s")

            # gen-1 silu partial: single-MM groups in banks freed by the
            # gen-2 drains, drained to the separate partial output.
            for i in range(G1):
                ps = ppool.tile([128, FDP], f32, tag="psum", name=f"psilu_{i}")
                ks = slice(i * FDP, (i + 1) * FDP)
                nc.tensor.matmul(
                    ps[:], w_sb[:, 8, :], silu_sb[:, ks], start=True, stop=True
                )
                ob = opool.tile([128, FDP], bf16, tag="ob", name=f"obs_{i}")
                if i % 2 == 0:
                    nc.vector.tensor_copy(ob[:], ps[:])
                else:
                    nc.scalar.copy(ob[:], ps[:])
                nc.scalar.dma_start(outs[:, ks], ob[:])

    nc.compile()
    return nc


def _prep_weights(coeffs, base_w):
    import ml_dtypes

    g = np.arange(GRID, dtype=np.float64)
    K = np.exp(7.0 * g - g * g)  # t_g = basis_g * e^(g^2-7g) -> fold inverse
    blocks = [
        (coeffs[:, :, gi].astype(np.float64) * K[gi]).T for gi in range(GRID)
    ]  # [in, out] each
    blocks.append(base_w.astype(np.float64).T)
    wtm = np.concatenate(blocks, axis=1)  # [128, 9*128]
    return np.ascontiguousarray(wtm.astype(ml_dtypes.bfloat16))


def _gather(results):
    """Merge per-core outputs: out + silu partial for the first G1 tiles."""
    cols = []
    for c in range(NCORES):
        full = results[c]["out"].astype(np.float32)  # [128, BLOC]
        part = results[c]["outs"].astype(np.float32)  # [128, G1*FDP]
        full[:, : G1 * FDP] += part
        cols.append(full)
    return np.ascontiguousarray(np.concatenate(cols, axis=1).T)


def kernel(x, coeffs, base_w, centers):
    from concourse.bass_utils import run_bass_kernel_spmd

    global _NC
    if _NC is None:
        _NC = _build()

    wtm = _prep_weights(coeffs, base_w)
    xT = np.ascontiguousarray(np.asarray(x, dtype=np.float32).T)  # [128, B]
    in_maps = [
        {
            "xt": np.ascontiguousarray(xT[:, c * BLOC : (c + 1) * BLOC]),
            "wt": wtm,
        }
        for c in range(NCORES)
    ]
    res = run_bass_kernel_spmd(_NC, in_maps, list(range(NCORES)))
    return _gather(res.results)
